# revision 21
# baseline (speedup 1.0000x reference)
"""JointsMSELoss with online hard-keypoint mining (top-k) on 8 TRN2 NeuronCores.

Strategy (data-parallel over batch, per sharding hint):
  - Full inputs: output/target [256, 17, 96, 72] f32, target_weight [256, 17, 1].
  - Shard batch 256 -> 8 cores x 32. Per core the shard is viewed flat as
    2176 rows of 1728 pixels (rows r' = (b*17 + j)*4 + s; s in 0..3 splits the
    6912-pixel joint map into 4 sub-rows), which tiles exactly into
    17 x [128 partitions, 1728].
  - Host interleaves o and t row-wise into one [2176, 3456] tensor so each
    tile is ONE contiguous 1.77MB DMA (the DVE TensorTensor encoding only
    supports a single embedded sync wait, so the subtract may only depend on
    one DMA).
  - Per tile, two DVE ops: d = o - t, then a fused scalar_tensor_tensor
    (d * 1.0) * d with accum_out = per-row sum(d^2).
  - Each core outputs [128, 17] per-row partial sums (8.7KB); host applies
    the per-(b,j) weight^2, reduces to per-joint sums across cores, computes
    mean losses, top-k of 17 values, and the final scalar. All heavy lifting
    (241MB of reads) is on device; host math is O(17k) floats.
"""

import os
import sys

for _p in ("/opt/trn_rl_repo", "/root/.axon_site/_ro/trn_rl_repo"):
    if os.path.isdir(_p) and _p not in sys.path:
        sys.path.insert(0, _p)

import numpy as np

import concourse.bass as bass
import concourse.tile as tile
from concourse import mybir
from concourse.bass_utils import run_bass_kernel_spmd

N_CORES = 8
B, J, H, W = 256, 17, 96, 72
PIX = H * W                 # 6912
B_LOC = B // N_CORES        # 32
ROWS = B_LOC * J            # 544 (b, j) rows per core
SPLIT = 4                   # sub-rows per (b, j) row -> 544*4 = 2176 = 17*128
RP = ROWS * SPLIT           # 2176 partition-rows
FREE = PIX // SPLIT         # 1728
NT = RP // 128              # 17 tiles of [128, 2*FREE]

F32 = mybir.dt.float32

_NC_CACHE = {}


NBUF = 8  # in-flight [o|t] tiles


def _build_nc():
    """Per-core program: stream interleaved [o|t] tiles, emit [128, NT]
    per-row sum((o-t)^2).

    Raw bass (not Tile): this walrus build only supports ONE embedded sync
    wait per compute/DMA instruction, and Tile's auto-semaphore pass emits
    two (slot-reuse WAR + DMA-lane FIFO). With explicit standalone wait_ge
    instructions every work instruction carries at most a then_inc.
    """
    import contextlib

    nc = bass.Bass()
    ot = nc.declare_dram_parameter("ot", [RP, 2 * FREE], F32, isOutput=False)
    out = nc.declare_dram_parameter("partial", [128, NT + 1], F32, isOutput=True)

    ot_tiles = ot[:].rearrange("(n p) f -> n p f", p=128)

    # split the first tile's load into partition chunks: descriptor generation
    # for the very first DMA is exposed (~2us for 128 rows), so start small
    FIRST_CHUNKS = 8

    with contextlib.ExitStack() as ctx:
        io = [
            ctx.enter_context(nc.sbuf_tensor(f"io{b}", [128, 2 * FREE], F32))
            for b in range(NBUF)
        ]
        d_pp = [
            ctx.enter_context(nc.sbuf_tensor(f"d{p}", [128, FREE], F32))
            for p in range(4)
        ]
        junk_pp = [
            ctx.enter_context(nc.sbuf_tensor(f"junk{p}", [128, FREE], F32))
            for p in range(2)
        ]
        sums = ctx.enter_context(nc.sbuf_tensor("sums", [128, NT + 1], F32))
        dma_sems = [
            ctx.enter_context(nc.semaphore(f"dma_sem{b}")) for b in range(NBUF)
        ]
        tt_sem = ctx.enter_context(nc.semaphore("tt_sem"))
        act_sem = ctx.enter_context(nc.semaphore("act_sem"))
        out_sem = ctx.enter_context(nc.semaphore("out_sem"))
        block = ctx.enter_context(nc.Block())

        # per-slot expected dma_sem value after the load of iteration i
        slot_val = [0] * NBUF
        dve_wait_val = [0] * NT

        # Compute sub-steps: tiles 0..NT-2 are one (SUB, SQUARE) pair; the
        # last tile is processed as two free-dim halves so the post-stream
        # drain (compute after the final DMA byte lands) is halved.
        # Each step: (tile i, col_lo, col_n, accum col) on d buffer i%2.
        # accum_out overwrites (it is not +=), so the two halves of the last
        # tile write separate columns NT-1 and NT; the host adds them.
        steps = []
        for i in range(NT - 1):
            steps.append((i, 0, FREE, i))
        half = FREE // 2
        steps.append((NT - 1, 0, half, NT - 1))
        steps.append((NT - 1, half, FREE - half, NT))

        @block.sync
        def _(sync):
            for i in range(NT):
                s = i % NBUF
                if i >= NBUF:
                    # slot free once SUB of iteration i-NBUF has consumed it
                    sync.wait_ge(tt_sem, i - NBUF + 1)
                if i == 0:
                    pstep = 128 // FIRST_CHUNKS
                    for c in range(FIRST_CHUNKS):
                        sync.dma_start(
                            io[s][c * pstep : (c + 1) * pstep, :],
                            ot_tiles[i][c * pstep : (c + 1) * pstep, :],
                        ).then_inc(dma_sems[s], 16)
                    slot_val[s] += 16 * FIRST_CHUNKS
                else:
                    sync.dma_start(io[s][:], ot_tiles[i]).then_inc(dma_sems[s], 16)
                    slot_val[s] += 16
                dve_wait_val[i] = slot_val[s]

        @block.vector
        def _(vector):
            for k, (i, lo, n, _col) in enumerate(steps):
                vector.wait_ge(dma_sems[i % NBUF], dve_wait_val[i])
                if k >= 4:
                    # d 4-deep rotation WAR: SQUARE of step k-4 must have
                    # read d_pp[k%4]; depth 4 hides the cross-engine hops
                    vector.wait_ge(act_sem, k - 3)
                buf = io[i % NBUF]
                nc.vector.tensor_sub(
                    d_pp[k % 4][:, :n],
                    buf[:, lo : lo + n],
                    buf[:, FREE + lo : FREE + lo + n],
                ).then_inc(tt_sem, 1)

        @block.scalar
        def _(scalar):
            for k, (i, lo, n, col) in enumerate(steps):
                scalar.wait_ge(tt_sem, k + 1)
                if k >= 2:
                    # junk ping-pong WAW: Square of step k-2 must have retired
                    scalar.wait_ge(act_sem, k - 1)
                # junk = d^2 ; sums[:, col] = sum_free(d^2)
                nc.scalar.activation(
                    junk_pp[k % 2][:, :n],
                    d_pp[k % 4][:, :n],
                    mybir.ActivationFunctionType.Square,
                    accum_out=sums[:, col : col + 1],
                ).then_inc(act_sem, 1)
            # out-DMA straight from the ACT sequencer's HWDGE ring: saves the
            # cross-engine sem hop at the very end of the kernel
            scalar.wait_ge(act_sem, len(steps))
            scalar.dma_start(out[:], sums[:]).then_inc(out_sem, 16)
            scalar.wait_ge(out_sem, 16)

    return nc


def _get_nc():
    if "nc" not in _NC_CACHE:
        _NC_CACHE["nc"] = _build_nc()
    return _NC_CACHE["nc"]


def _make_in_maps(output, target):
    output = np.asarray(output, dtype=np.float32)
    target = np.asarray(target, dtype=np.float32)

    in_maps = []
    for c in range(N_CORES):
        sl = slice(c * B_LOC, (c + 1) * B_LOC)
        ot = np.empty((RP, 2 * FREE), dtype=np.float32)
        ot[:, :FREE] = output[sl].reshape(RP, FREE)
        ot[:, FREE:] = target[sl].reshape(RP, FREE)
        in_maps.append({"ot": ot})
    return in_maps


def _finish_on_host(partials, target_weight, top_k):
    """partials: list of [128, NT+1] per-core sum(d^2) -> final scalar loss.

    Columns 0..NT-1 are per-row-tile sums; column NT holds the second half of
    the last tile (fold it into column NT-1)."""
    target_weight = np.asarray(target_weight, dtype=np.float64)
    per_j_total = np.zeros(J, dtype=np.float64)
    for c, p in enumerate(partials):
        p = np.asarray(p, dtype=np.float64)
        p = np.concatenate(
            [p[:, : NT - 1], (p[:, NT - 1] + p[:, NT])[:, None]], axis=1
        )
        srow = p.T.ravel()                                  # [2176], r' order
        per_bj = srow.reshape(ROWS, SPLIT).sum(axis=1)      # [544] sum(d^2)
        tw = target_weight[c * B_LOC : (c + 1) * B_LOC].reshape(ROWS)
        per_bj *= tw * tw                                   # apply weight^2
        per_j_total += per_bj.reshape(B_LOC, J).sum(axis=0) # [17]
    losses = per_j_total / float(B * PIX)                   # per-joint MSE
    k = int(top_k)
    topk_vals = np.sort(losses)[::-1][:k]
    return np.float32(topk_vals.sum() / k)


def kernel(output, target, target_weight, top_k=8):
    nc = _get_nc()
    in_maps = _make_in_maps(output, target)
    res = run_bass_kernel_spmd(nc, in_maps, core_ids=list(range(N_CORES)))
    partials = [res.results[c]["partial"] for c in range(N_CORES)]
    return _finish_on_host(partials, target_weight, top_k)


def run_profiled(output, target, target_weight, top_k=8):
    """Like kernel(), but captures an NTFF profile; returns (loss, exec_time_ns)."""
    nc = _get_nc()
    in_maps = _make_in_maps(output, target)
    res = run_bass_kernel_spmd(
        nc, in_maps, core_ids=list(range(N_CORES)), trace=True
    )
    partials = [res.results[c]["partial"] for c in range(N_CORES)]
    return _finish_on_host(partials, target_weight, top_k), res.exec_time_ns


# revision 22
# speedup vs baseline: 1.0576x; 1.0576x over previous
"""JointsMSELoss with online hard-keypoint mining (top-k) on 8 TRN2 NeuronCores.

Strategy (data-parallel over batch, per sharding hint):
  - Full inputs: output/target [256, 17, 96, 72] f32, target_weight [256, 17, 1].
  - Shard batch 256 -> 8 cores x 32. Per core the shard is viewed flat as
    2176 rows of 1728 pixels (rows r' = (b*17 + j)*4 + s; s in 0..3 splits the
    6912-pixel joint map into 4 sub-rows), which tiles exactly into
    17 x [128 partitions, 1728].
  - Host interleaves o and t row-wise into one [2176, 3456] tensor so each
    tile is ONE contiguous 1.77MB DMA (the DVE TensorTensor encoding only
    supports a single embedded sync wait, so the subtract may only depend on
    one DMA).
  - Per tile, two DVE ops: d = o - t, then a fused scalar_tensor_tensor
    (d * 1.0) * d with accum_out = per-row sum(d^2).
  - Each core outputs [128, 17] per-row partial sums (8.7KB); host applies
    the per-(b,j) weight^2, reduces to per-joint sums across cores, computes
    mean losses, top-k of 17 values, and the final scalar. All heavy lifting
    (241MB of reads) is on device; host math is O(17k) floats.
"""

import os
import sys

for _p in ("/opt/trn_rl_repo", "/root/.axon_site/_ro/trn_rl_repo"):
    if os.path.isdir(_p) and _p not in sys.path:
        sys.path.insert(0, _p)

import numpy as np

import concourse.bass as bass
import concourse.tile as tile
from concourse import mybir
from concourse.bass_utils import run_bass_kernel_spmd

N_CORES = 8
B, J, H, W = 256, 17, 96, 72
PIX = H * W                 # 6912
B_LOC = B // N_CORES        # 32
ROWS = B_LOC * J            # 544 (b, j) rows per core
SPLIT = 4                   # sub-rows per (b, j) row -> 544*4 = 2176 = 17*128
RP = ROWS * SPLIT           # 2176 partition-rows
FREE = PIX // SPLIT         # 1728
NT = RP // 128              # 17 tiles of [128, 2*FREE]

F32 = mybir.dt.float32

_NC_CACHE = {}


NBUF = 6  # in-flight [o|t] tiles


def _build_nc():
    """Per-core program: stream interleaved [o|t] tiles, emit [128, NT]
    per-row sum((o-t)^2).

    Raw bass (not Tile): this walrus build only supports ONE embedded sync
    wait per compute/DMA instruction, and Tile's auto-semaphore pass emits
    two (slot-reuse WAR + DMA-lane FIFO). With explicit standalone wait_ge
    instructions every work instruction carries at most a then_inc.
    """
    import contextlib

    nc = bass.Bass()
    ot = nc.declare_dram_parameter("ot", [RP, 2 * FREE], F32, isOutput=False)
    out = nc.declare_dram_parameter("partial", [128, NT + 1], F32, isOutput=True)

    ot_tiles = ot[:].rearrange("(n p) f -> n p f", p=128)

    # split the first tile's load into partition chunks: descriptor generation
    # for the very first DMA is exposed (~2us for 128 rows), so start small
    FIRST_CHUNKS = 4

    with contextlib.ExitStack() as ctx:
        io = [
            ctx.enter_context(nc.sbuf_tensor(f"io{b}", [128, 2 * FREE], F32))
            for b in range(NBUF)
        ]
        d_pp = [
            ctx.enter_context(nc.sbuf_tensor(f"d{p}", [128, FREE], F32))
            for p in range(4)
        ]
        junk_pp = [
            ctx.enter_context(nc.sbuf_tensor(f"junk{p}", [128, FREE], F32))
            for p in range(2)
        ]
        sums = ctx.enter_context(nc.sbuf_tensor("sums", [128, NT + 1], F32))
        dma_sems = [
            ctx.enter_context(nc.semaphore(f"dma_sem{b}")) for b in range(NBUF)
        ]
        tt_sem = ctx.enter_context(nc.semaphore("tt_sem"))
        act_sem = ctx.enter_context(nc.semaphore("act_sem"))
        out_sem = ctx.enter_context(nc.semaphore("out_sem"))
        block = ctx.enter_context(nc.Block())

        # per-slot expected dma_sem value after the load of iteration i
        slot_val = [0] * NBUF
        dve_wait_val = [0] * NT

        # Compute sub-steps: tiles 0..NT-2 are one (SUB, SQUARE) pair; the
        # last tile is processed as two free-dim halves so the post-stream
        # drain (compute after the final DMA byte lands) is halved.
        # Each step: (tile i, col_lo, col_n, accum col) on d buffer i%2.
        # accum_out overwrites (it is not +=), so the two halves of the last
        # tile write separate columns NT-1 and NT; the host adds them.
        steps = []
        for i in range(NT - 1):
            steps.append((i, 0, FREE, i))
        half = FREE // 2
        steps.append((NT - 1, 0, half, NT - 1))
        steps.append((NT - 1, half, FREE - half, NT))

        @block.sync
        def _(sync):
            for i in range(NT):
                s = i % NBUF
                if i >= NBUF:
                    # slot free once SUB of iteration i-NBUF has consumed it
                    sync.wait_ge(tt_sem, i - NBUF + 1)
                if i == 0:
                    pstep = 128 // FIRST_CHUNKS
                    for c in range(FIRST_CHUNKS):
                        sync.dma_start(
                            io[s][c * pstep : (c + 1) * pstep, :],
                            ot_tiles[i][c * pstep : (c + 1) * pstep, :],
                        ).then_inc(dma_sems[s], 16)
                    slot_val[s] += 16 * FIRST_CHUNKS
                else:
                    sync.dma_start(io[s][:], ot_tiles[i]).then_inc(dma_sems[s], 16)
                    slot_val[s] += 16
                dve_wait_val[i] = slot_val[s]

        @block.vector
        def _(vector):
            for k, (i, lo, n, _col) in enumerate(steps):
                vector.wait_ge(dma_sems[i % NBUF], dve_wait_val[i])
                if k >= 4:
                    # d 4-deep rotation WAR: SQUARE of step k-4 must have
                    # read d_pp[k%4]; depth 4 hides the cross-engine hops
                    vector.wait_ge(act_sem, k - 3)
                buf = io[i % NBUF]
                nc.vector.tensor_sub(
                    d_pp[k % 4][:, :n],
                    buf[:, lo : lo + n],
                    buf[:, FREE + lo : FREE + lo + n],
                ).then_inc(tt_sem, 1)

        @block.scalar
        def _(scalar):
            for k, (i, lo, n, col) in enumerate(steps):
                scalar.wait_ge(tt_sem, k + 1)
                if k >= 2:
                    # junk ping-pong WAW: Square of step k-2 must have retired
                    scalar.wait_ge(act_sem, k - 1)
                # junk = d^2 ; sums[:, col] = sum_free(d^2)
                nc.scalar.activation(
                    junk_pp[k % 2][:, :n],
                    d_pp[k % 4][:, :n],
                    mybir.ActivationFunctionType.Square,
                    accum_out=sums[:, col : col + 1],
                ).then_inc(act_sem, 1)
            # out-DMA straight from the ACT sequencer's HWDGE ring: saves the
            # cross-engine sem hop at the very end of the kernel
            scalar.wait_ge(act_sem, len(steps))
            scalar.dma_start(out[:], sums[:]).then_inc(out_sem, 16)
            scalar.wait_ge(out_sem, 16)

    return nc


def _get_nc():
    if "nc" not in _NC_CACHE:
        _NC_CACHE["nc"] = _build_nc()
    return _NC_CACHE["nc"]


def _make_in_maps(output, target):
    output = np.asarray(output, dtype=np.float32)
    target = np.asarray(target, dtype=np.float32)

    in_maps = []
    for c in range(N_CORES):
        sl = slice(c * B_LOC, (c + 1) * B_LOC)
        ot = np.empty((RP, 2 * FREE), dtype=np.float32)
        ot[:, :FREE] = output[sl].reshape(RP, FREE)
        ot[:, FREE:] = target[sl].reshape(RP, FREE)
        in_maps.append({"ot": ot})
    return in_maps


def _finish_on_host(partials, target_weight, top_k):
    """partials: list of [128, NT+1] per-core sum(d^2) -> final scalar loss.

    Columns 0..NT-1 are per-row-tile sums; column NT holds the second half of
    the last tile (fold it into column NT-1)."""
    target_weight = np.asarray(target_weight, dtype=np.float64)
    per_j_total = np.zeros(J, dtype=np.float64)
    for c, p in enumerate(partials):
        p = np.asarray(p, dtype=np.float64)
        p = np.concatenate(
            [p[:, : NT - 1], (p[:, NT - 1] + p[:, NT])[:, None]], axis=1
        )
        srow = p.T.ravel()                                  # [2176], r' order
        per_bj = srow.reshape(ROWS, SPLIT).sum(axis=1)      # [544] sum(d^2)
        tw = target_weight[c * B_LOC : (c + 1) * B_LOC].reshape(ROWS)
        per_bj *= tw * tw                                   # apply weight^2
        per_j_total += per_bj.reshape(B_LOC, J).sum(axis=0) # [17]
    losses = per_j_total / float(B * PIX)                   # per-joint MSE
    k = int(top_k)
    topk_vals = np.sort(losses)[::-1][:k]
    return np.float32(topk_vals.sum() / k)


def kernel(output, target, target_weight, top_k=8):
    nc = _get_nc()
    in_maps = _make_in_maps(output, target)
    res = run_bass_kernel_spmd(nc, in_maps, core_ids=list(range(N_CORES)))
    partials = [res.results[c]["partial"] for c in range(N_CORES)]
    return _finish_on_host(partials, target_weight, top_k)


def run_profiled(output, target, target_weight, top_k=8):
    """Like kernel(), but captures an NTFF profile; returns (loss, exec_time_ns)."""
    nc = _get_nc()
    in_maps = _make_in_maps(output, target)
    res = run_bass_kernel_spmd(
        nc, in_maps, core_ids=list(range(N_CORES)), trace=True
    )
    partials = [res.results[c]["partial"] for c in range(N_CORES)]
    return _finish_on_host(partials, target_weight, top_k), res.exec_time_ns
